# revision 17
# baseline (speedup 1.0000x reference)
"""Trainium2 Bass kernel for nn_AttentionBlock (dense_cnn).

Computes, per batch b:
    a = sigmoid(MLP(x))              # per-pixel 2048->64->16->8->1 w/ ReLU
    out[b] = sum_p(a*x) / sum_p(a)   # weighted GAP over 14x14 pixels

Sharding: pure data parallelism over batch (B=64) across 8 NeuronCores
(8 batches/core); weights replicated; no cross-core communication.

Per-core strategy (x shard = [1568, 2048], pixel tiles of 128):
  Phase 1: the MLP chain runs transposed (channels on partitions) in bf16
    over 512-pixel super-tiles; x^T arrives via plain DMA from a
    host-pretransposed bf16 layout (per-super contiguous blocks).
    ReLU+bias fuse into the PSUM->SBUF copy on the scalar engine.
    Pre-sigmoid attention logits collect into one [128, NT] buffer.
  One batched sigmoid produces all attention weights.
  Phase 2: GAP runs as PE matmuls with stationary masked-A [pix, 8
    batches] and the natural-layout x tile (resident in SBUF) as the
    moving operand, accumulating [8, 2048] in PSUM over all pixel tiles.
    mean/mean == sum/sum; the final scale by 1/sum(a) fuses into the
    PSUM->SBUF copy.
  The GAP path runs in float32r (near-fp32 PE streaming); the bf16 chain
  only perturbs the attention weights (~6e-4 scale-relative measured).
  GAP_BF16 switches the GAP path to bf16 (halves DMA + GAP PE time,
  ~1e-3 scale-relative).
"""

import ml_dtypes
import numpy as np
from contextlib import ExitStack

from concourse import bacc, mybir, tile
from concourse.bass_utils import run_bass_kernel_spmd

F32 = mybir.dt.float32
AF = mybir.ActivationFunctionType

GAP_BF16 = False
DT = mybir.dt.bfloat16 if GAP_BF16 else mybir.dt.float32r
DT_NP = ml_dtypes.bfloat16 if GAP_BF16 else np.float32
BF = mybir.dt.bfloat16       # MLP chain path

B, HH, WW, C = 64, 14, 14, 2048
NCORES = 8
BPC = B // NCORES            # 8 batches per core
PIX = HH * WW                # 196 pixels per batch
NPIX = BPC * PIX             # 1568 pixels per core
P = 128
NCH = C // P                 # 16 channel chunks
D1, D2, D3 = 64, 16, 8

# flat pixel tiles (GAP granularity, real pixels)
TILES = [(t * P, min(P, NPIX - t * P)) for t in range((NPIX + P - 1) // P)]
NT = len(TILES)
NPIX_PAD = NT * P            # 1664: chain processes zero-padded pixels
# super-tiles (MLP chain granularity; <=512 wide so PSUM stays in-bank)
_SPLIT = [4, 4, 4, 1]
SUPER = []
_t0 = 0
for _n in _SPLIT:
    SUPER.append(list(range(_t0, _t0 + _n)))
    _t0 += _n
XT_COLS = NCH * NPIX_PAD


def build_program(b4_val: float):
    nc = bacc.Bacc("TRN2", target_bir_lowering=False, debug=False)

    x_d = nc.dram_tensor("x", [NPIX, C], DT, kind="ExternalInput")
    xt_d = nc.dram_tensor("xt", [P, XT_COLS], BF, kind="ExternalInput")
    w1_d = nc.dram_tensor("W1r", [P, NCH, D1], BF, kind="ExternalInput")
    w2_d = nc.dram_tensor("W2", [D1, D2], BF, kind="ExternalInput")
    w3_d = nc.dram_tensor("W3", [D2, D3], BF, kind="ExternalInput")
    w4_d = nc.dram_tensor("W4", [D3, 2], BF, kind="ExternalInput")
    b1_d = nc.dram_tensor("b1c", [D1, 1], F32, kind="ExternalInput")
    b2_d = nc.dram_tensor("b2c", [D2, 1], F32, kind="ExternalInput")
    b3_d = nc.dram_tensor("b3c", [D3, 1], F32, kind="ExternalInput")
    one_d = nc.dram_tensor("ones", [P, 2], DT, kind="ExternalInput")
    msk_d = nc.dram_tensor("mask", [P, NT, BPC], DT, kind="ExternalInput")
    out_d = nc.dram_tensor("out", [BPC, C], F32, kind="ExternalOutput")

    with tile.TileContext(nc) as tc, ExitStack() as ctx:
        const = ctx.enter_context(tc.tile_pool(name="const", bufs=1))
        acc = ctx.enter_context(tc.tile_pool(name="acc", bufs=1))
        xpool = ctx.enter_context(tc.tile_pool(name="xin", bufs=NT))
        xtp = ctx.enter_context(tc.tile_pool(name="xT", bufs=2))
        hpool = ctx.enter_context(tc.tile_pool(name="hsb", bufs=2))
        misc = ctx.enter_context(tc.tile_pool(name="misc", bufs=3))
        ps_chain = ctx.enter_context(tc.tile_pool(name="chain", bufs=3, space="PSUM"))
        ps_gap = ctx.enter_context(tc.tile_pool(name="gap", bufs=1, space="PSUM"))

        # ---- constants (scalar HWDGE queue; x/xT stream on sync) ----
        w1_sb = const.tile([P, NCH, D1], BF)
        nc.scalar.dma_start(w1_sb[:], w1_d[:])
        w2_sb = const.tile([D1, D2], BF)
        nc.scalar.dma_start(w2_sb[:], w2_d[:])
        w3_sb = const.tile([D2, D3], BF)
        nc.scalar.dma_start(w3_sb[:], w3_d[:])
        w4_sb = const.tile([D3, 2], BF)
        nc.scalar.dma_start(w4_sb[:], w4_d[:])
        b1_sb = const.tile([D1, 1], F32)
        nc.scalar.dma_start(b1_sb[:], b1_d[:])
        b2_sb = const.tile([D2, 1], F32)
        nc.scalar.dma_start(b2_sb[:], b2_d[:])
        b3_sb = const.tile([D3, 1], F32)
        nc.scalar.dma_start(b3_sb[:], b3_d[:])
        ones = const.tile([P, 2], DT)
        nc.scalar.dma_start(ones[:], one_d[:])
        mask = const.tile([P, NT, BPC], DT)
        nc.scalar.dma_start(mask[:], msk_d[:])

        # ---- accumulators ----
        gap_ps = ps_gap.tile([BPC, 4, 512], F32)      # [8, 2048] over 4 banks
        cnt_sb = acc.tile([BPC, 1], F32)
        nc.vector.memset(cnt_sb[:], 0.0)
        a_pre = acc.tile([P, NT], F32)                # pre-sigmoid logits

        # ---- phase 1: MLP chain over super-tiles ----
        xts = {}
        for tlist in SUPER:
            s_off = TILES[tlist[0]][0]
            s_sz = P * len(tlist)

            # natural-layout x tiles (GAP path), resident until phase 2
            for t in tlist:
                off, sz = TILES[t]
                xt = xpool.tile([sz, C], DT, tag="x")
                nc.sync.dma_start(xt[:], x_d[off:off + sz, :])
                xts[t] = xt

            # xT (channels on partitions), host-pretransposed contiguous block
            xT = xtp.tile([P, NCH, s_sz], BF, tag="xT")
            nc.sync.dma_start(
                xT[:].rearrange("p k s -> p (k s)"),
                xt_d[:, NCH * s_off:NCH * (s_off + s_sz)])

            h1_ps = ps_chain.tile([D1, s_sz], F32, tag="chain")
            for k in range(NCH):
                nc.tensor.matmul(
                    h1_ps[:], w1_sb[:, k, :], xT[:, k, :],
                    start=(k == 0), stop=(k == NCH - 1),
                )
            h1_sb = hpool.tile([D1, s_sz], BF, tag="h1")
            nc.scalar.activation(h1_sb[:], h1_ps[:], AF.Relu, bias=b1_sb[:])

            h2_ps = ps_chain.tile([D2, s_sz], F32, tag="chain")
            nc.tensor.matmul(h2_ps[:], w2_sb[:], h1_sb[:], start=True, stop=True)
            h2_sb = hpool.tile([D2, s_sz], BF, tag="h2")
            nc.scalar.activation(h2_sb[:], h2_ps[:], AF.Relu, bias=b2_sb[:])

            h3_ps = ps_chain.tile([D3, s_sz], F32, tag="chain")
            nc.tensor.matmul(h3_ps[:], w3_sb[:], h2_sb[:], start=True, stop=True)
            h3_sb = hpool.tile([D3, s_sz], BF, tag="h3")
            nc.scalar.activation(h3_sb[:], h3_ps[:], AF.Relu, bias=b3_sb[:])

            for i, t in enumerate(tlist):
                i0 = i * P
                a_ps = ps_chain.tile([P, 2], F32, tag="chain")
                nc.tensor.matmul(a_ps[:], h3_sb[:, i0:i0 + P], w4_sb[:],
                                 start=True, stop=True)
                nc.vector.tensor_copy(a_pre[:, t:t + 1], a_ps[:, 0:1])

        # ---- batched sigmoid: all attention weights at once ----
        a_all = acc.tile([P, NT], DT)
        nc.scalar.activation(a_all[:], a_pre[:], AF.Sigmoid, bias=b4_val)

        # ---- phase 2: masked-A GAP over resident x tiles ----
        last_t = NT - 1
        for t, (off, sz) in enumerate(TILES):
            A = misc.tile([sz, BPC], DT, tag="A")
            nc.vector.tensor_mul(A[:], a_all[0:sz, t:t + 1].to_broadcast([sz, BPC]),
                                 mask[0:sz, t, :])
            for n in range(4):
                nc.tensor.matmul(
                    gap_ps[:, n, :], A[:],
                    xts[t][:, n * 512:(n + 1) * 512],
                    start=(t == 0), stop=(t == last_t),
                )
            cnt_ps = ps_chain.tile([BPC, 2], F32, tag="chain")
            nc.tensor.matmul(cnt_ps[:], A[:], ones[0:sz, :],
                             start=True, stop=True)
            nc.vector.tensor_add(cnt_sb[:], cnt_sb[:], cnt_ps[:, 0:1])

        # ---- finalize: out = gap_sum / cnt ----
        recip = acc.tile([BPC, 1], F32)
        nc.vector.reciprocal(recip[:], cnt_sb[:])
        out_sb = acc.tile([BPC, C], F32)
        for n in range(4):
            nc.scalar.activation(out_sb[:, n * 512:(n + 1) * 512],
                                 gap_ps[:, n, :], AF.Copy, scale=recip[:])
        nc.sync.dma_start(out_d[:], out_sb[:])

    nc.compile()
    return nc


def _make_mask():
    m = np.zeros((P, NT, BPC), dtype=np.float32)
    for t, (off, sz) in enumerate(TILES):
        for p in range(sz):
            m[p, t, (off + p) // PIX] = 1.0
    return m


def make_in_maps(x, W1, b1, W2, b2, W3, b3, W4, b4):
    x = np.ascontiguousarray(np.asarray(x, dtype=np.float32))
    base = {
        "W1r": np.ascontiguousarray(
            np.asarray(W1, np.float32).reshape(NCH, P, D1).transpose(1, 0, 2)
            .astype(ml_dtypes.bfloat16)),
        "W2": np.ascontiguousarray(np.asarray(W2, ml_dtypes.bfloat16)),
        "W3": np.ascontiguousarray(np.asarray(W3, ml_dtypes.bfloat16)),
        "W4": np.ascontiguousarray(np.concatenate(
            [np.asarray(W4, np.float32),
             np.zeros((D3, 1), np.float32)], axis=1).astype(ml_dtypes.bfloat16)),
        "b1c": np.asarray(b1, np.float32).reshape(D1, 1).copy(),
        "b2c": np.asarray(b2, np.float32).reshape(D2, 1).copy(),
        "b3c": np.asarray(b3, np.float32).reshape(D3, 1).copy(),
        "ones": np.ones((P, 2), dtype=DT_NP),
        "mask": _make_mask().astype(DT_NP),
    }
    xs = x.reshape(B, PIX, C)
    maps = []
    for c in range(NCORES):
        xc = np.ascontiguousarray(xs[c * BPC:(c + 1) * BPC].reshape(NPIX, C))
        xcp = np.zeros((NPIX_PAD, C), dtype=np.float32)
        xcp[:NPIX] = xc
        # [P, NCH, NPIX_PAD] -> per-super contiguous [P, XT_COLS]
        xct3 = xcp.T.reshape(NCH, P, NPIX_PAD).transpose(1, 0, 2)
        blocks = []
        for tlist in SUPER:
            s_off = TILES[tlist[0]][0]
            s_sz = P * len(tlist)
            blocks.append(xct3[:, :, s_off:s_off + s_sz].reshape(P, -1))
        xct = np.ascontiguousarray(
            np.concatenate(blocks, axis=1)).astype(ml_dtypes.bfloat16)
        maps.append({"x": xc.astype(DT_NP), "xt": xct, **base})
    return maps


def kernel(x, W1, b1, W2, b2, W3, b3, W4, b4, _profile=False, **_ignored):
    nc = build_program(float(np.asarray(b4, np.float32).reshape(-1)[0]))
    in_maps = make_in_maps(x, W1, b1, W2, b2, W3, b3, W4, b4)
    res = run_bass_kernel_spmd(nc, in_maps, core_ids=list(range(NCORES)),
                               trace=_profile)
    out = np.concatenate([res.results[c]["out"] for c in range(NCORES)], axis=0)
    out = np.ascontiguousarray(out.astype(np.float32))
    if _profile:
        return out, res
    return out


# revision 18
# speedup vs baseline: 1.3786x; 1.3786x over previous
"""Trainium2 Bass kernel for nn_AttentionBlock (dense_cnn).

Computes, per batch b:
    a = sigmoid(MLP(x))              # per-pixel 2048->64->16->8->1 w/ ReLU
    out[b] = sum_p(a*x) / sum_p(a)   # weighted GAP over 14x14 pixels

Sharding: pure data parallelism over batch (B=64) across 8 NeuronCores
(8 batches/core); weights replicated; no cross-core communication.

Per-core strategy (x shard = [1568, 2048], pixel tiles of 128):
  Phase 1: the MLP chain runs transposed (channels on partitions) in bf16
    over 512-pixel super-tiles; x^T arrives via plain DMA from a
    host-pretransposed bf16 layout (per-super contiguous blocks).
    ReLU+bias fuse into the PSUM->SBUF copy on the scalar engine.
    Pre-sigmoid attention logits collect into one [128, NT] buffer.
  One batched sigmoid produces all attention weights.
  Phase 2: GAP runs as PE matmuls with stationary masked-A [pix, 8
    batches] and the natural-layout x tile (resident in SBUF) as the
    moving operand, accumulating [8, 2048] in PSUM over all pixel tiles.
    mean/mean == sum/sum; the final scale by 1/sum(a) fuses into the
    PSUM->SBUF copy.
  The GAP path runs in float32r (near-fp32 PE streaming); the bf16 chain
  only perturbs the attention weights (~6e-4 scale-relative measured).
  GAP_BF16 switches the GAP path to bf16 (halves DMA + GAP PE time,
  ~1e-3 scale-relative).
"""

import ml_dtypes
import numpy as np
from contextlib import ExitStack

from concourse import bacc, mybir, tile
from concourse.bass_utils import run_bass_kernel_spmd

F32 = mybir.dt.float32
AF = mybir.ActivationFunctionType

GAP_BF16 = False
DT = mybir.dt.bfloat16 if GAP_BF16 else mybir.dt.float32r
DT_NP = ml_dtypes.bfloat16 if GAP_BF16 else np.float32
BF = mybir.dt.bfloat16       # MLP chain path

B, HH, WW, C = 64, 14, 14, 2048
NCORES = 8
BPC = B // NCORES            # 8 batches per core
PIX = HH * WW                # 196 pixels per batch
NPIX = BPC * PIX             # 1568 pixels per core
P = 128
NCH = C // P                 # 16 channel chunks
D1, D2, D3 = 64, 16, 8

# flat pixel tiles (GAP granularity, real pixels)
TILES = [(t * P, min(P, NPIX - t * P)) for t in range((NPIX + P - 1) // P)]
NT = len(TILES)
NPIX_PAD = NT * P            # 1664: chain processes zero-padded pixels
# super-tiles (MLP chain granularity; <=512 wide so PSUM stays in-bank)
_SPLIT = [4, 4, 4, 1]
SUPER = []
_t0 = 0
for _n in _SPLIT:
    SUPER.append(list(range(_t0, _t0 + _n)))
    _t0 += _n
XT_COLS = NCH * NPIX_PAD


def build_program(b4_val: float):
    nc = bacc.Bacc("TRN2", target_bir_lowering=False, debug=False)

    x_d = nc.dram_tensor("x", [NPIX, C], DT, kind="ExternalInput")
    xt_d = nc.dram_tensor("xt", [P, XT_COLS], BF, kind="ExternalInput")
    w1_d = nc.dram_tensor("W1r", [P, NCH, D1], BF, kind="ExternalInput")
    w2_d = nc.dram_tensor("W2", [D1, D2], BF, kind="ExternalInput")
    w3_d = nc.dram_tensor("W3", [D2, D3], BF, kind="ExternalInput")
    w4_d = nc.dram_tensor("W4", [D3, 2], BF, kind="ExternalInput")
    b1_d = nc.dram_tensor("b1c", [D1, 1], F32, kind="ExternalInput")
    b2_d = nc.dram_tensor("b2c", [D2, 1], F32, kind="ExternalInput")
    b3_d = nc.dram_tensor("b3c", [D3, 1], F32, kind="ExternalInput")
    one_d = nc.dram_tensor("ones", [P, 2], DT, kind="ExternalInput")
    msk_d = nc.dram_tensor("mask", [P, NT, BPC], DT, kind="ExternalInput")
    out_d = nc.dram_tensor("out", [BPC, C], F32, kind="ExternalOutput")

    with tile.TileContext(nc) as tc, ExitStack() as ctx:
        const = ctx.enter_context(tc.tile_pool(name="const", bufs=1))
        acc = ctx.enter_context(tc.tile_pool(name="acc", bufs=1))
        xpool = ctx.enter_context(tc.tile_pool(name="xin", bufs=NT))
        xtp = ctx.enter_context(tc.tile_pool(name="xT", bufs=2))
        hpool = ctx.enter_context(tc.tile_pool(name="hsb", bufs=2))
        misc = ctx.enter_context(tc.tile_pool(name="misc", bufs=3))
        ps_chain = ctx.enter_context(tc.tile_pool(name="chain", bufs=3, space="PSUM"))
        ps_gap = ctx.enter_context(tc.tile_pool(name="gap", bufs=1, space="PSUM"))

        # ---- constants (scalar HWDGE queue; x/xT stream on sync) ----
        w1_sb = const.tile([P, NCH, D1], BF)
        nc.scalar.dma_start(w1_sb[:], w1_d[:])
        w2_sb = const.tile([D1, D2], BF)
        nc.scalar.dma_start(w2_sb[:], w2_d[:])
        w3_sb = const.tile([D2, D3], BF)
        nc.scalar.dma_start(w3_sb[:], w3_d[:])
        w4_sb = const.tile([D3, 2], BF)
        nc.scalar.dma_start(w4_sb[:], w4_d[:])
        b1_sb = const.tile([D1, 1], F32)
        nc.scalar.dma_start(b1_sb[:], b1_d[:])
        b2_sb = const.tile([D2, 1], F32)
        nc.scalar.dma_start(b2_sb[:], b2_d[:])
        b3_sb = const.tile([D3, 1], F32)
        nc.scalar.dma_start(b3_sb[:], b3_d[:])
        ones = const.tile([P, 2], DT)
        nc.scalar.dma_start(ones[:], one_d[:])
        mask = const.tile([P, NT, BPC], DT)
        nc.scalar.dma_start(mask[:], msk_d[:])

        # ---- accumulators ----
        gap_ps = ps_gap.tile([BPC, 4, 512], F32)      # [8, 2048] over 4 banks
        cnt_sb = acc.tile([BPC, 1], F32)
        nc.vector.memset(cnt_sb[:], 0.0)

        last_t = NT - 1
        # ---- single pass: MLP chain + sigmoid + GAP per super-tile ----
        for tlist in SUPER:
            s_off = TILES[tlist[0]][0]
            s_sz = P * len(tlist)
            nts = len(tlist)

            # xT (channels on partitions), host-pretransposed contiguous
            # block -- issued first: the chain is the critical consumer
            xT = xtp.tile([P, NCH, s_sz], BF, tag="xT")
            nc.sync.dma_start(
                xT[:].rearrange("p k s -> p (k s)"),
                xt_d[:, NCH * s_off:NCH * (s_off + s_sz)])

            # natural-layout x tiles (GAP path)
            xts = {}
            for t in tlist:
                off, sz = TILES[t]
                xt = xpool.tile([sz, C], DT, tag="x")
                nc.sync.dma_start(xt[:], x_d[off:off + sz, :])
                xts[t] = xt

            h1_ps = ps_chain.tile([D1, s_sz], F32, tag="chain")
            for k in range(NCH):
                nc.tensor.matmul(
                    h1_ps[:], w1_sb[:, k, :], xT[:, k, :],
                    start=(k == 0), stop=(k == NCH - 1),
                )
            h1_sb = hpool.tile([D1, s_sz], BF, tag="h1")
            nc.scalar.activation(h1_sb[:], h1_ps[:], AF.Relu, bias=b1_sb[:])

            h2_ps = ps_chain.tile([D2, s_sz], F32, tag="chain")
            nc.tensor.matmul(h2_ps[:], w2_sb[:], h1_sb[:], start=True, stop=True)
            h2_sb = hpool.tile([D2, s_sz], BF, tag="h2")
            nc.scalar.activation(h2_sb[:], h2_ps[:], AF.Relu, bias=b2_sb[:])

            h3_ps = ps_chain.tile([D3, s_sz], F32, tag="chain")
            nc.tensor.matmul(h3_ps[:], w3_sb[:], h2_sb[:], start=True, stop=True)
            h3_sb = hpool.tile([D3, s_sz], BF, tag="h3")
            nc.scalar.activation(h3_sb[:], h3_ps[:], AF.Relu, bias=b3_sb[:])

            a_pre = misc.tile([P, nts], F32, tag="apre")
            for i, t in enumerate(tlist):
                i0 = i * P
                a_ps = ps_chain.tile([P, 2], F32, tag="chain")
                nc.tensor.matmul(a_ps[:], h3_sb[:, i0:i0 + P], w4_sb[:],
                                 start=True, stop=True)
                nc.vector.tensor_copy(a_pre[:, i:i + 1], a_ps[:, 0:1])

            a_s = misc.tile([P, nts], DT, tag="as")
            nc.scalar.activation(a_s[:], a_pre[:], AF.Sigmoid, bias=b4_val)

            for i, t in enumerate(tlist):
                off, sz = TILES[t]
                A = misc.tile([sz, BPC], DT, tag="A")
                nc.vector.tensor_mul(
                    A[:], a_s[0:sz, i:i + 1].to_broadcast([sz, BPC]),
                    mask[0:sz, t, :])
                for n in range(4):
                    nc.tensor.matmul(
                        gap_ps[:, n, :], A[:],
                        xts[t][:, n * 512:(n + 1) * 512],
                        start=(t == 0), stop=(t == last_t),
                    )
                cnt_ps = ps_chain.tile([BPC, 2], F32, tag="chain")
                nc.tensor.matmul(cnt_ps[:], A[:], ones[0:sz, :],
                                 start=True, stop=True)
                nc.vector.tensor_add(cnt_sb[:], cnt_sb[:], cnt_ps[:, 0:1])

        # ---- finalize: out = gap_sum / cnt ----
        recip = acc.tile([BPC, 1], F32)
        nc.vector.reciprocal(recip[:], cnt_sb[:])
        out_sb = acc.tile([BPC, C], F32)
        for n in range(4):
            nc.scalar.activation(out_sb[:, n * 512:(n + 1) * 512],
                                 gap_ps[:, n, :], AF.Copy, scale=recip[:])
        nc.sync.dma_start(out_d[:], out_sb[:])

    nc.compile()
    return nc


def _make_mask():
    m = np.zeros((P, NT, BPC), dtype=np.float32)
    for t, (off, sz) in enumerate(TILES):
        for p in range(sz):
            m[p, t, (off + p) // PIX] = 1.0
    return m


def make_in_maps(x, W1, b1, W2, b2, W3, b3, W4, b4):
    x = np.ascontiguousarray(np.asarray(x, dtype=np.float32))
    base = {
        "W1r": np.ascontiguousarray(
            np.asarray(W1, np.float32).reshape(NCH, P, D1).transpose(1, 0, 2)
            .astype(ml_dtypes.bfloat16)),
        "W2": np.ascontiguousarray(np.asarray(W2, ml_dtypes.bfloat16)),
        "W3": np.ascontiguousarray(np.asarray(W3, ml_dtypes.bfloat16)),
        "W4": np.ascontiguousarray(np.concatenate(
            [np.asarray(W4, np.float32),
             np.zeros((D3, 1), np.float32)], axis=1).astype(ml_dtypes.bfloat16)),
        "b1c": np.asarray(b1, np.float32).reshape(D1, 1).copy(),
        "b2c": np.asarray(b2, np.float32).reshape(D2, 1).copy(),
        "b3c": np.asarray(b3, np.float32).reshape(D3, 1).copy(),
        "ones": np.ones((P, 2), dtype=DT_NP),
        "mask": _make_mask().astype(DT_NP),
    }
    xs = x.reshape(B, PIX, C)
    maps = []
    for c in range(NCORES):
        xc = np.ascontiguousarray(xs[c * BPC:(c + 1) * BPC].reshape(NPIX, C))
        xcp = np.zeros((NPIX_PAD, C), dtype=np.float32)
        xcp[:NPIX] = xc
        # [P, NCH, NPIX_PAD] -> per-super contiguous [P, XT_COLS]
        xct3 = xcp.T.reshape(NCH, P, NPIX_PAD).transpose(1, 0, 2)
        blocks = []
        for tlist in SUPER:
            s_off = TILES[tlist[0]][0]
            s_sz = P * len(tlist)
            blocks.append(xct3[:, :, s_off:s_off + s_sz].reshape(P, -1))
        xct = np.ascontiguousarray(
            np.concatenate(blocks, axis=1)).astype(ml_dtypes.bfloat16)
        maps.append({"x": xc.astype(DT_NP), "xt": xct, **base})
    return maps


def kernel(x, W1, b1, W2, b2, W3, b3, W4, b4, _profile=False, **_ignored):
    nc = build_program(float(np.asarray(b4, np.float32).reshape(-1)[0]))
    in_maps = make_in_maps(x, W1, b1, W2, b2, W3, b3, W4, b4)
    res = run_bass_kernel_spmd(nc, in_maps, core_ids=list(range(NCORES)),
                               trace=_profile)
    out = np.concatenate([res.results[c]["out"] for c in range(NCORES)], axis=0)
    out = np.ascontiguousarray(out.astype(np.float32))
    if _profile:
        return out, res
    return out
